# revision 1
# baseline (speedup 1.0000x reference)
"""Trainium2 Bass kernel for nn_BDHHRM_23424751632641 (sparse_attention).

Contract: kernel(**inputs) takes FULL unsharded numpy inputs (as produced by
the problem's setup_inputs) and returns the FULL (B, T, V) fp32 logits.

Sharding: 8 cores = 4 batches x 2 head-groups. Core c handles batch c//2 and
heads [4*(c%2), 4*(c%2)+4). The decoder GEMM partial products are summed with
a pairwise AllReduce ([[0,1],[2,3],[4,5],[6,7]]); everything else is local.

Device numerics: all GEMMs run as float32r (full-rate fp32, ~1.5e-4 rel err),
vector/LN math in fp32, RoPE tables resident in bf16.
"""

import math
import sys
import types

import numpy as np

sys.path.insert(0, "/opt/trn_rl_repo")

import ml_dtypes  # noqa: E402

import concourse.bass as bass  # noqa: E402
import concourse.tile as tile  # noqa: E402
from concourse import bacc, mybir, bass_utils  # noqa: E402
from concourse.masks import make_identity  # noqa: E402

# Problem shapes (hardcoded per contract)
B, T, D, V = 4, 512, 512, 8192
NH_TOT, N = 8, 4096
NH = 4          # heads per core
NCORES = 8
P = 128
TP = T // P     # 4 t-tiles
DP = D // P     # 4 d-tiles
NT = N // P     # 32 n-tiles per head
NCHUNK = 8      # chunks per head; chunk = 2 even + 2 odd n-tiles (512 latents)
NE = NT // 2    # 16 even tiles
H_CYCLES, L_CYCLES = 3, 2
EPS = 1e-5
SCALE = float(N) ** -0.5

F32 = mybir.dt.float32
F32R = mybir.dt.float32r
BF16 = mybir.dt.bfloat16
ADD = mybir.AluOpType.add
SUB = mybir.AluOpType.subtract
MUL = mybir.AluOpType.mult
AF = mybir.ActivationFunctionType

BLOCK_SEQ = ["L", "L", "H"] * 3
SM_BASE = [0, 512, 896, 1152]  # packed col base for scores row-tile k (width 512-128k)


def _dram_in(nc, name, shape, dt=F32):
    return nc.dram_tensor(name, list(shape), dt, kind="ExternalInput").ap()


def build_program(n_blocks=9, debug_z=False):
    """Build the per-core device program. Returns (nc, meta)."""
    nc = bacc.Bacc("TRN2", target_bir_lowering=False, debug=False, num_devices=NCORES)

    emb = _dram_in(nc, "emb", (T, D))
    zh0 = _dram_in(nc, "zh0", (T, D))
    zl0 = _dram_in(nc, "zl0", (T, D))
    w_enc = {k: _dram_in(nc, f"{k}_enc", (NH, D, N)) for k in ("l", "h")}
    w_encv = {k: _dram_in(nc, f"{k}_encv", (NH, D, N)) for k in ("l", "h")}
    w_dec = {k: _dram_in(nc, f"{k}_dec", (NH * N, D)) for k in ("l", "h")}
    ch_d = _dram_in(nc, "ch", (NE * P, T), BF16)
    sh_d = _dram_in(nc, "sh", (NE * P, T), BF16)
    umask_d = _dram_in(nc, "umask", (P, P))
    wlm_d = _dram_in(nc, "wlm", (D, V // 2))

    if debug_z:
        z_out = nc.dram_tensor("z_out", [T, D], F32, kind="ExternalOutput").ap()
    else:
        logits = nc.dram_tensor("logits", [T, V // 2], F32, kind="ExternalOutput").ap()

    with tile.TileContext(nc) as tc:
        with (
            tc.tile_pool(name="pers", bufs=1) as pers,
            tc.tile_pool(name="qrp", bufs=2) as qrp,
            tc.tile_pool(name="wp", bufs=2) as wp,
            tc.tile_pool(name="ysp", bufs=2) as yspp,
            tc.tile_pool(name="rtmp", bufs=2) as rtmpp,
            tc.tile_pool(name="stat", bufs=8) as statp,
            tc.tile_pool(name="ps", bufs=8, space="PSUM") as psp,
            tc.tile_pool(name="dram", bufs=2, space="DRAM") as dramp,
        ):
            # ---- persistent tiles ----
            ch_t = pers.tile([P, NE, T], BF16, tag="ch")
            sh_t = pers.tile([P, NE, T], BF16, tag="sh")
            z = {
                "L": pers.tile([P, TP, D], F32R, tag="zl", name="z_L"),
                "H": pers.tile([P, TP, D], F32R, tag="zh", name="z_H"),
            }
            inp = pers.tile([P, TP, D], BF16, tag="inp")
            xsp = pers.tile([P, NT, T], F32R, tag="xsp")
            xi = pers.tile([P, TP, D], F32R, tag="xi")
            xiT = pers.tile([P, DP, T], F32R, tag="xiT")
            ykv = pers.tile([P, TP, D], F32R, tag="ykv")
            ykvT = pers.tile([P, DP, T], F32R, tag="ykvT")
            y_acc = pers.tile([P, TP, D], F32, tag="yacc")
            # packed strict-upper scores: row-tile k occupies cols [SM_BASE[k], SM_BASE[k]+512-128k)
            s_mask = pers.tile([P, 1280], F32R, tag="smask")
            ident32 = pers.tile([P, P], F32, tag="ident32")
            ident = pers.tile([P, P], F32R, tag="ident")
            umask = pers.tile([P, P], F32, tag="umask")
            epsv = pers.tile([P, 1], F32, tag="epsv")
            nc.gpsimd.memset(epsv[:], EPS)

            make_identity(nc, ident32)
            nc.vector.tensor_copy(ident[:], ident32[:])
            nc.sync.dma_start(umask[:], umask_d)
            nc.sync.dma_start(ch_t[:], ch_d.rearrange("(i p) t -> p i t", p=P))
            nc.sync.dma_start(sh_t[:], sh_d.rearrange("(i p) t -> p i t", p=P))
            nc.sync.dma_start(z["H"][:], zh0.rearrange("(i p) d -> p i d", p=P).bitcast(F32R))
            nc.sync.dma_start(z["L"][:], zl0.rearrange("(i p) d -> p i d", p=P).bitcast(F32R))

            # ---- helpers ----
            def ln(src_fn, dst_fn, n_tiles=TP):
                """dst[i] = layernorm(src[i]) along free dim (512)."""
                for i in range(n_tiles):
                    st6 = statp.tile([P, 6], F32, tag="st6")
                    st2 = statp.tile([P, 2], F32, tag="st2")
                    std = statp.tile([P, 1], F32, tag="std")
                    rstd = statp.tile([P, 1], F32, tag="rstd")
                    mb = statp.tile([P, 1], F32, tag="mb")
                    nc.vector.bn_stats(st6[:], src_fn(i))
                    nc.vector.bn_aggr(st2[:], st6[:])
                    nc.scalar.activation(std[:], st2[:, 1:2], AF.Sqrt, bias=epsv[:])
                    nc.vector.reciprocal(rstd[:], std[:])
                    nc.vector.scalar_tensor_tensor(mb[:], st2[:, 0:1], -1.0, rstd[:], MUL, MUL)
                    nc.scalar.activation(dst_fn(i), src_fn(i), AF.Identity, bias=mb[:], scale=rstd[:])

            def transpose_into(src_tile, dst_tile):
                """dst[p, dj, t] = src[t, d] transposed; both [P, 4, 512]."""
                for i in range(TP):
                    for dj in range(DP):
                        pt = psp.tile([P, P], F32R, tag="ps", name=f"tp_{i}_{dj}")
                        nc.tensor.transpose(pt[:], src_tile[:, i, dj * P:(dj + 1) * P], ident[:])
                        nc.scalar.activation(dst_tile[:, dj, i * P:(i + 1) * P], pt[:], AF.Copy)

            # ---- embedding prologue: inp = LN(emb) (stage emb via y_acc) ----
            nc.sync.dma_start(y_acc[:], emb.rearrange("(i p) d -> p i d", p=P))
            ln(lambda i: y_acc[:, i, :], lambda i: inp[:, i, :])

            # ---- block ----
            def run_block(kind, bi):
                lk = kind.lower()
                enc_d, encv_d, dec_d = w_enc[lk], w_encv[lk], w_dec[lk]
                x = z[kind]
                other = z["H" if kind == "L" else "L"]

                # tmp = x + inj  (inj = z_other + inp for L, z_other for H)
                tmp = ykvT  # reuse (dead at block start)
                for i in range(TP):
                    nc.vector.tensor_tensor(tmp[:, i, :], x[:, i, :], other[:, i, :], ADD)
                    if kind == "L":
                        nc.vector.tensor_tensor(tmp[:, i, :], tmp[:, i, :], inp[:, i, :], ADD)
                # xi = LN(tmp)
                ln(lambda i: tmp[:, i, :], lambda i: xi[:, i, :])
                transpose_into(xi, xiT)

                for h in range(NH):
                    # ---- A/B/C: x_sparse, rope, scores ----
                    sc = [psp.tile([P, 512], F32, tag="ps", name=f"sc{bi}_{h}_{i}") for i in range(TP)]
                    for c in range(NCHUNK):
                        wenc = wp.tile([P, DP, 512], F32R, tag="w")
                        src = enc_d[h].rearrange("(dt p) n -> p dt n", p=P)
                        nc.sync.dma_start(wenc[:, :, 0:256], src[:, :, 256 * c:256 * c + 256].bitcast(F32R))
                        nc.sync.dma_start(wenc[:, :, 256:512],
                                          src[:, :, 2048 + 256 * c:2048 + 256 * c + 256].bitcast(F32R))
                        for j in range(4):
                            jj = 2 * c + j if j < 2 else NE + 2 * c + (j - 2)
                            px = psp.tile([P, 512], F32, tag="ps", name=f"px{bi}_{h}_{c}_{j}")
                            for kt in range(DP):
                                nc.tensor.matmul(px[:], wenc[:, kt, j * P:(j + 1) * P], xiT[:, kt, :],
                                                 start=(kt == 0), stop=(kt == DP - 1))
                            nc.scalar.activation(xsp[:, jj, :], px[:], AF.Relu)
                        qrt = qrp.tile([P, 4, T], F32R, tag="qr")
                        e2 = xsp[:, 2 * c:2 * c + 2, :]
                        o2 = xsp[:, NE + 2 * c:NE + 2 * c + 2, :]
                        ch2 = ch_t[:, 2 * c:2 * c + 2, :]
                        sh2 = sh_t[:, 2 * c:2 * c + 2, :]
                        t1 = rtmpp.tile([P, 2, T], BF16, tag="rt")
                        t2 = rtmpp.tile([P, 2, T], BF16, tag="rt")
                        nc.gpsimd.tensor_tensor(t1[:], o2, sh2, MUL)
                        nc.vector.tensor_tensor(qrt[:, 0:2, :], e2, ch2, MUL)
                        nc.vector.tensor_tensor(qrt[:, 0:2, :], qrt[:, 0:2, :], t1[:], SUB)
                        nc.gpsimd.tensor_tensor(t2[:], e2, sh2, MUL)
                        nc.vector.tensor_tensor(qrt[:, 2:4, :], o2, ch2, MUL)
                        nc.vector.tensor_tensor(qrt[:, 2:4, :], qrt[:, 2:4, :], t2[:], ADD)
                        for i in range(TP):
                            # row i=3 is computed at width 256 (from t=256, first 128
                            # discarded): f32r moving dim <256 runs at 4 cyc/row.
                            t_lo = min(i * P, 256)
                            w_sc = 512 - t_lo
                            for k in range(4):
                                nc.tensor.matmul(sc[i][:, :w_sc],
                                                 qrt[:, k, i * P:(i + 1) * P],
                                                 qrt[:, k, t_lo:],
                                                 start=(c == 0 and k == 0),
                                                 stop=(c == NCHUNK - 1 and k == 3),
                                                 skip_group_check=True)
                    # ---- C-finish: masked scaled copy into packed s_mask ----
                    for i in range(TP):
                        w_i = 512 - P * i
                        d_off = P * i - min(i * P, 256)  # col offset of t=128*i inside sc[i]
                        nc.vector.tensor_tensor(s_mask[:, SM_BASE[i]:SM_BASE[i] + P],
                                                sc[i][:, d_off:d_off + P], umask[:], MUL)
                        if i < TP - 1:
                            nc.scalar.activation(s_mask[:, SM_BASE[i] + P:SM_BASE[i] + w_i],
                                                 sc[i][:, d_off + P:d_off + w_i],
                                                 AF.Copy, scale=SCALE)
                    # ---- D: attn = S_masked @ xi ; E: ykv = LN(attn) ----
                    for i in range(TP):
                        pa = psp.tile([P, 512], F32, tag="ps", name=f"pa{bi}_{h}_{i}")
                        for k in range(i + 1):
                            off = SM_BASE[k] + P * (i - k)
                            nc.tensor.matmul(pa[:], s_mask[:, off:off + P], xi[:, k, :],
                                             start=(k == 0), stop=(k == i))
                        st6 = statp.tile([P, 6], F32, tag="st6")
                        st2 = statp.tile([P, 2], F32, tag="st2")
                        std = statp.tile([P, 1], F32, tag="std")
                        rstd = statp.tile([P, 1], F32, tag="rstd")
                        mb = statp.tile([P, 1], F32, tag="mb")
                        nc.vector.bn_stats(st6[:], pa[:])
                        nc.vector.bn_aggr(st2[:], st6[:])
                        nc.scalar.activation(std[:], st2[:, 1:2], AF.Sqrt, bias=epsv[:])
                        nc.vector.reciprocal(rstd[:], std[:])
                        nc.vector.scalar_tensor_tensor(mb[:], st2[:, 0:1], -1.0, rstd[:], MUL, MUL)
                        nc.scalar.activation(ykv[:, i, :], pa[:], AF.Identity, bias=mb[:], scale=rstd[:])
                    transpose_into(ykv, ykvT)

                    # ---- F/G/H: y_sparse, xy, dec partial ----
                    ydec = [psp.tile([P, 512], F32, tag="ps", name=f"yd{bi}_{h}_{i}") for i in range(TP)]
                    for c in range(NCHUNK):
                        wencv = wp.tile([P, DP, 512], F32R, tag="w")
                        srcv = encv_d[h].rearrange("(dt p) n -> p dt n", p=P)
                        nc.sync.dma_start(wencv[:, :, 0:256], srcv[:, :, 256 * c:256 * c + 256].bitcast(F32R))
                        nc.sync.dma_start(wencv[:, :, 256:512],
                                          srcv[:, :, 2048 + 256 * c:2048 + 256 * c + 256].bitcast(F32R))
                        for jpair in range(2):
                            jj0 = 2 * c if jpair == 0 else NE + 2 * c
                            yst = yspp.tile([P, 2, T], F32, tag="ys")
                            for j2 in range(2):
                                j = 2 * jpair + j2
                                py = psp.tile([P, 512], F32, tag="ps", name=f"py{bi}_{h}_{c}_{j}")
                                for kt in range(DP):
                                    nc.tensor.matmul(py[:], wencv[:, kt, j * P:(j + 1) * P], ykvT[:, kt, :],
                                                     start=(kt == 0), stop=(kt == DP - 1))
                                nc.scalar.activation(yst[:, j2, :], py[:], AF.Relu)
                            nc.vector.tensor_tensor(xsp[:, jj0:jj0 + 2, :], xsp[:, jj0:jj0 + 2, :],
                                                    yst[:], MUL)
                        wdec = wp.tile([P, DP, 512], F32R, tag="w")
                        base = h * N
                        nc.sync.dma_start(
                            wdec[:, 0:2, :],
                            dec_d[base + 256 * c:base + 256 * c + 256, :]
                            .rearrange("(kt p) d -> p kt d", p=P).bitcast(F32R))
                        nc.sync.dma_start(
                            wdec[:, 2:4, :],
                            dec_d[base + 2048 + 256 * c:base + 2048 + 256 * c + 256, :]
                            .rearrange("(kt p) d -> p kt d", p=P).bitcast(F32R))
                        for i in range(TP):
                            for k in range(4):
                                jjk = 2 * c + k if k < 2 else NE + 2 * c + (k - 2)
                                nc.tensor.matmul(ydec[i][:], xsp[:, jjk, i * P:(i + 1) * P], wdec[:, k, :],
                                                 start=(c == 0 and k == 0),
                                                 stop=(c == NCHUNK - 1 and k == 3),
                                                 skip_group_check=True)
                    for i in range(TP):
                        if h == 0:
                            nc.vector.tensor_copy(y_acc[:, i, :], ydec[i][:])
                        else:
                            nc.vector.tensor_tensor(y_acc[:, i, :], y_acc[:, i, :], ydec[i][:], ADD)

                # ---- pairwise AllGather of y partials + local add ----
                bi_in = dramp.tile([T, D], F32, tag="arin")
                bi_out = dramp.tile([2 * T, D], F32, tag="arout")
                nc.gpsimd.dma_start(bi_in.rearrange("(i p) d -> p i d", p=P), y_acc[:])
                nc.gpsimd.collective_compute(
                    "AllGather", mybir.AluOpType.bypass,
                    replica_groups=[[0, 1], [2, 3], [4, 5], [6, 7]],
                    ins=[bi_in.opt()],
                    outs=[bi_out.opt()],
                )
                yhalf = ykv  # reuse as landing for the peer half
                nc.sync.dma_start(y_acc[:], bi_out[0:T].rearrange("(i p) d -> p i d", p=P))
                nc.sync.dma_start(yhalf[:], bi_out[T:2 * T].rearrange("(i p) d -> p i d", p=P).bitcast(F32R))
                for i in range(TP):
                    nc.vector.tensor_tensor(y_acc[:, i, :], y_acc[:, i, :], yhalf[:, i, :], ADD)

                # ---- yn = LN(y_sum); newz = LN(x + yn) ----
                yn = ykv  # reuse
                ln(lambda i: y_acc[:, i, :], lambda i: yn[:, i, :])
                tmp3 = ykvT  # reuse
                for i in range(TP):
                    nc.vector.tensor_tensor(tmp3[:, i, :], x[:, i, :], yn[:, i, :], ADD)
                ln(lambda i: tmp3[:, i, :], lambda i: x[:, i, :])

            for bi in range(n_blocks):
                run_block(BLOCK_SEQ[bi], bi)

            if debug_z:
                last = z[BLOCK_SEQ[n_blocks - 1]]
                nc.sync.dma_start(z_out.rearrange("(i p) d -> p i d", p=P), last.bitcast(F32)[:])
            else:
                # ---- lm_head: logits = z_H @ wlm (this core's V/2 slice) ----
                zhT = xiT  # reuse
                transpose_into(z["H"], zhT)
                for vc in range(8):
                    wlm_t = wp.tile([P, DP, 512], F32R, tag="w")
                    nc.sync.dma_start(
                        wlm_t[:],
                        wlm_d[:, 512 * vc:512 * (vc + 1)]
                        .rearrange("(kt p) v -> p kt v", p=P).bitcast(F32R))
                    for i in range(TP):
                        pl = psp.tile([P, 512], F32, tag="ps", name=f"pl{vc}_{i}")
                        for kt in range(DP):
                            nc.tensor.matmul(pl[:], zhT[:, kt, i * P:(i + 1) * P], wlm_t[:, kt, :],
                                             start=(kt == 0), stop=(kt == DP - 1))
                        ot = yspp.tile([P, T], F32, tag="ys", name=f"lm_{vc}_{i}")
                        nc.scalar.activation(ot[:], pl[:], AF.Copy)
                        nc.sync.dma_start(logits[i * P:(i + 1) * P, 512 * vc:512 * (vc + 1)], ot[:])

    nc.finalize()
    return nc


# ---------------- host side ----------------

_PERM = np.concatenate([np.arange(0, N, 2), np.arange(1, N, 2)])


def _rope_half_tables():
    """bf16 (2048, T) cos/sin tables, pair-deduped, transposed to [i, t]."""
    q = np.floor(np.arange(N, dtype=np.float32) / 2.0) * 2.0
    freqs = (1.0 / np.power(np.float32(2.0 ** 16), q / np.float32(N))
             / np.float32(2 * math.pi)).astype(np.float32)
    f_even = freqs[0::2]  # (2048,)
    phases = np.arange(T, dtype=np.float32)[None, :] * f_even[:, None]  # (2048, T)
    ang = (phases % 1.0) * np.float32(2 * math.pi)
    ch = np.cos(ang).astype(ml_dtypes.bfloat16)
    sh = np.sin(ang).astype(ml_dtypes.bfloat16)
    return ch, sh


def make_in_maps(inputs):
    idx = np.asarray(inputs["idx"])
    embed_w = np.asarray(inputs["embed_w"], np.float32)
    lm_head = np.asarray(inputs["lm_head"], np.float32)
    h_init = np.asarray(inputs["h_init"], np.float32)
    l_init = np.asarray(inputs["l_init"], np.float32)

    ch, sh = _rope_half_tables()
    umask = (np.arange(P)[:, None] < np.arange(P)[None, :]).astype(np.float32) * SCALE

    w = {}
    for k in ("l", "h"):
        enc = np.asarray(inputs[f"{k}_enc"], np.float32)[:, :, _PERM]
        encv = np.asarray(inputs[f"{k}_enc_v"], np.float32)[:, :, _PERM]
        dec = np.asarray(inputs[f"{k}_dec"], np.float32).reshape(NH_TOT, N, D)[:, _PERM, :]
        w[k] = (enc, encv, dec)

    zh0 = np.ascontiguousarray(np.broadcast_to(h_init, (T, D)), dtype=np.float32)
    zl0 = np.ascontiguousarray(np.broadcast_to(l_init, (T, D)), dtype=np.float32)

    in_maps = []
    for c in range(NCORES):
        b, g = c // 2, c % 2
        hs = slice(4 * g, 4 * g + 4)
        emb = np.ascontiguousarray(embed_w[idx[b]], dtype=np.float32)
        m = {
            "emb": emb,
            "zh0": zh0,
            "zl0": zl0,
            "ch": np.ascontiguousarray(ch),
            "sh": np.ascontiguousarray(sh),
            "umask": umask,
            "wlm": np.ascontiguousarray(lm_head[:, g * (V // 2):(g + 1) * (V // 2)]),
        }
        for k in ("l", "h"):
            enc, encv, dec = w[k]
            m[f"{k}_enc"] = np.ascontiguousarray(enc[hs])
            m[f"{k}_encv"] = np.ascontiguousarray(encv[hs])
            m[f"{k}_dec"] = np.ascontiguousarray(dec[hs].reshape(NH * N, D))
        in_maps.append(m)
    return in_maps


_CACHE = {}


def _get_program(n_blocks=9, debug_z=False):
    key = (n_blocks, debug_z)
    if key not in _CACHE:
        _CACHE[key] = build_program(n_blocks, debug_z)
    return _CACHE[key]


def run(inputs, n_blocks=9, debug_z=False, trace=False, tmpdir=None):
    nc = _get_program(n_blocks, debug_z)
    in_maps = make_in_maps(inputs)
    r = bass_utils.run_bass_kernel_spmd(
        nc, in_maps, core_ids=list(range(NCORES)), trace=trace, tmpdir=tmpdir)
    return r


def kernel(**inputs) -> np.ndarray:
    r = run(inputs)
    out = np.empty((B, T, V), np.float32)
    for c in range(NCORES):
        b, g = c // 2, c % 2
        out[b][:, g * (V // 2):(g + 1) * (V // 2)] = r.results[c]["logits"]
    return out



# revision 6
# speedup vs baseline: 1.6712x; 1.6712x over previous
"""Trainium2 Bass kernel for nn_BDHHRM_23424751632641 (sparse_attention).

Contract: kernel(**inputs) takes FULL unsharded numpy inputs (as produced by
the problem's setup_inputs) and returns the FULL (B, T, V) fp32 logits.

Sharding: 8 cores = 4 batches x 2 head-groups. Core c handles batch c//2 and
heads [4*(c%2), 4*(c%2)+4). The decoder GEMM partial sums are combined with a
pairwise AllReduce ([[0,1],[2,3],[4,5],[6,7]]); everything else is local.

Device numerics: all GEMM operands in bf16 (fp32 PSUM accumulation), LN /
residual stream in fp32. End-to-end rel err ~1e-2 (sim-verified).

Schedule: software-pipelined head loop. Per head h the emission order is
  C-finish(h) -> D+LN(h) -> transpose(h) -> A_pre(h+1) -> F/H(h) -> flush(h)
  -> A+S interleaved rest(h+1)
so the PE fills LN/collective gaps of head h with head h+1's encoder GEMMs.
x_sparse is double-buffered across heads; scores PSUM accumulators (4 banks)
and decoder accumulators (4 banks) are never live simultaneously.
"""

import math
import sys

import numpy as np

sys.path.insert(0, "/opt/trn_rl_repo")

import ml_dtypes  # noqa: E402

import concourse.bass as bass  # noqa: E402
import concourse.tile as tile  # noqa: E402
from concourse import bacc, mybir, bass_utils  # noqa: E402
from concourse.masks import make_identity  # noqa: E402

# Problem shapes (hardcoded per contract)
B, T, D, V = 4, 512, 512, 8192
NH_TOT, N = 8, 4096
NH = 4          # heads per core
NCORES = 8
P = 128
TP = T // P     # 4 t-tiles
DP = D // P     # 4 d-tiles
NT = N // P     # 32 n-tiles per head
NCH = 8         # chunks per head; chunk c = tiles [4c,4c+4) = (e,e,o,o)
H_CYCLES, L_CYCLES = 3, 2
EPS = 1e-5
SCALE = float(N) ** -0.5

F32 = mybir.dt.float32
BF16 = mybir.dt.bfloat16
ADD = mybir.AluOpType.add
SUB = mybir.AluOpType.subtract
MUL = mybir.AluOpType.mult
AF = mybir.ActivationFunctionType

BLOCK_SEQ = ["L", "L", "H"] * 3
SM_BASE = [0, 512, 896, 1152]  # packed col base for score row-tile k (width 512-128k)
SM_W = 1280


def _dram_in(nc, name, shape, dt=F32):
    return nc.dram_tensor(name, list(shape), dt, kind="ExternalInput").ap()


def build_program(n_blocks=9, debug_z=False):
    nc = bacc.Bacc("TRN2", target_bir_lowering=False, debug=False, num_devices=NCORES)

    emb = _dram_in(nc, "emb", (T, D))
    zh0 = _dram_in(nc, "zh0", (T, D))
    zl0 = _dram_in(nc, "zl0", (T, D))
    w_enc = {k: _dram_in(nc, f"{k}_enc", (NH, D, N), BF16) for k in ("l", "h")}
    w_encv = {k: _dram_in(nc, f"{k}_encv", (NH, D, N), BF16) for k in ("l", "h")}
    w_dec = {k: _dram_in(nc, f"{k}_dec", (NH * N, D), BF16) for k in ("l", "h")}
    ch_d = _dram_in(nc, "ch", (N // 2, T), BF16)
    sh_d = _dram_in(nc, "sh", (N // 2, T), BF16)
    umask_d = _dram_in(nc, "umask", (P, P))
    wlm_d = _dram_in(nc, "wlm", (D, V // 2), BF16)

    if debug_z:
        z_out = nc.dram_tensor("z_out", [T, D], F32, kind="ExternalOutput").ap()
    else:
        logits = nc.dram_tensor("logits", [T, V // 2], F32, kind="ExternalOutput").ap()

    with tile.TileContext(nc) as tc:
        with (
            tc.tile_pool(name="pers", bufs=1) as pers,
            tc.tile_pool(name="xspp", bufs=2) as xspp,
            tc.tile_pool(name="qrtp", bufs=4) as qrtp,
            tc.tile_pool(name="rtp", bufs=2) as rtp,
            tc.tile_pool(name="wp", bufs=3) as wp,
            tc.tile_pool(name="ystp", bufs=2) as ystp,
            tc.tile_pool(name="stat", bufs=8) as statp,
            tc.tile_pool(name="ps", bufs=8, space="PSUM") as psp,
            tc.tile_pool(name="dram", bufs=1, space="DRAM") as dramp,
        ):
            # ---- persistent tiles ----
            ch_t = pers.tile([P, NT // 2, T], BF16, tag="ch")
            sh_t = pers.tile([P, NT // 2, T], BF16, tag="sh")
            z = {
                "L": pers.tile([P, TP, D], F32, tag="zl", name="z_L"),
                "H": pers.tile([P, TP, D], F32, tag="zh", name="z_H"),
            }
            inp = pers.tile([P, TP, D], BF16, tag="inp")
            xi = pers.tile([P, TP, D], BF16, tag="xi")
            xiT = pers.tile([P, DP, T], BF16, tag="xiT")
            ykv = pers.tile([P, TP, D], BF16, tag="ykv")
            ykvT = pers.tile([P, DP, T], BF16, tag="ykvT")
            y_acc = pers.tile([P, TP, D], F32, tag="yacc")
            yn = pers.tile([P, TP, D], BF16, tag="yn")
            s_mask = pers.tile([P, SM_W], BF16, tag="smask")
            ident = pers.tile([P, P], BF16, tag="ident")
            umask = pers.tile([P, P], F32, tag="umask")
            epsv = pers.tile([P, 1], F32, tag="epsv")
            nc.gpsimd.memset(epsv[:], EPS)

            make_identity(nc, ident)
            nc.sync.dma_start(umask[:], umask_d)
            nc.sync.dma_start(ch_t[:], ch_d.rearrange("(i p) t -> p i t", p=P))
            nc.sync.dma_start(sh_t[:], sh_d.rearrange("(i p) t -> p i t", p=P))
            nc.sync.dma_start(z["H"][:], zh0.rearrange("(i p) d -> p i d", p=P))
            nc.sync.dma_start(z["L"][:], zl0.rearrange("(i p) d -> p i d", p=P))

            # ---- helpers ----
            def ln_tile(src_ap, dst_ap):
                """dst = layernorm(src) along free dim (D)."""
                st6 = statp.tile([P, 6], F32, tag="st6")
                st2 = statp.tile([P, 2], F32, tag="st2")
                std = statp.tile([P, 1], F32, tag="std")
                rstd = statp.tile([P, 1], F32, tag="rstd")
                mb = statp.tile([P, 1], F32, tag="mb")
                nc.vector.bn_stats(st6[:], src_ap)
                nc.vector.bn_aggr(st2[:], st6[:])
                nc.scalar.activation(std[:], st2[:, 1:2], AF.Sqrt, bias=epsv[:])
                nc.vector.reciprocal(rstd[:], std[:])
                nc.vector.scalar_tensor_tensor(mb[:], st2[:, 0:1], -1.0, rstd[:], MUL, MUL)
                nc.scalar.activation(dst_ap, src_ap, AF.Identity, bias=mb[:], scale=rstd[:])

            def transpose_tile(src_tile, dst_tile, i):
                """dst[:, dj, i*P:(i+1)*P] = src[:, i, :].T for dj in 0..DP."""
                for dj in range(DP):
                    pt = psp.tile([P, P], BF16, tag="ps", name=f"tp_{i}_{dj}")
                    nc.tensor.transpose(pt[:], src_tile[:, i, dj * P:(dj + 1) * P], ident[:])
                    nc.scalar.activation(dst_tile[:, dj, i * P:(i + 1) * P], pt[:], AF.Copy)

            # ---- embedding prologue: inp = LN(emb) ----
            nc.sync.dma_start(y_acc[:], emb.rearrange("(i p) d -> p i d", p=P))
            for i in range(TP):
                ln_tile(y_acc[:, i, :], inp[:, i, :])

            # ---- per-head phase emitters ----
            def A_chunks(st, chunks, tsplit=False):
                """x_sparse GEMM + relu + rope for the given chunks."""
                h, bi, enc_d = st["h"], st["bi"], st["enc"]
                xsp = st["xsp"]
                src = enc_d[h].rearrange("(dt p) n -> p dt n", p=P)
                for c in chunks:
                    w = wp.tile([P, DP, 512], BF16, tag="w")
                    nc.sync.dma_start(w[:], src[:, :, 512 * c:512 * (c + 1)])
                    for j in range(4):
                        px = psp.tile([P, 512], F32, tag="ps", name=f"px{bi}_{h}_{c}_{j}")
                        if tsplit:
                            for it in range(TP):
                                for kt in range(DP):
                                    nc.tensor.matmul(
                                        px[:, it * P:(it + 1) * P],
                                        w[:, kt, j * P:(j + 1) * P],
                                        xiT[:, kt, it * P:(it + 1) * P],
                                        start=(kt == 0), stop=(kt == DP - 1),
                                        skip_group_check=True)
                        else:
                            for kt in range(DP):
                                nc.tensor.matmul(px[:], w[:, kt, j * P:(j + 1) * P],
                                                 xiT[:, kt, :],
                                                 start=(kt == 0), stop=(kt == DP - 1))
                        nc.scalar.activation(xsp[:, 4 * c + j, :], px[:], AF.Relu)
                    qrt = qrtp.tile([P, 4, T], BF16, tag="qrt")
                    st["qrt"][c] = qrt
                    e2 = xsp[:, 4 * c:4 * c + 2, :]
                    o2 = xsp[:, 4 * c + 2:4 * c + 4, :]
                    ch2 = ch_t[:, 2 * c:2 * c + 2, :]
                    sh2 = sh_t[:, 2 * c:2 * c + 2, :]
                    t1 = rtp.tile([P, 2, T], BF16, tag="rt")
                    t2 = rtp.tile([P, 2, T], BF16, tag="rt")
                    nc.vector.tensor_tensor(t1[:], o2, sh2, MUL)
                    nc.vector.tensor_tensor(qrt[:, 0:2, :], e2, ch2, MUL)
                    nc.vector.tensor_tensor(qrt[:, 0:2, :], qrt[:, 0:2, :], t1[:], SUB)
                    nc.vector.tensor_tensor(t2[:], e2, sh2, MUL)
                    nc.vector.tensor_tensor(qrt[:, 2:4, :], o2, ch2, MUL)
                    nc.vector.tensor_tensor(qrt[:, 2:4, :], qrt[:, 2:4, :], t2[:], ADD)

            def S_chunk(st, c):
                """Scores contribution of chunk c into the 4 packed accumulators."""
                qrt = st["qrt"].pop(c)
                sc = st["sc"]
                for i in range(TP):
                    t_lo = P * i
                    w_sc = T - t_lo
                    for k in range(4):
                        nc.tensor.matmul(sc[i][:, :w_sc],
                                         qrt[:, k, t_lo:t_lo + P],
                                         qrt[:, k, t_lo:],
                                         start=(c == 0 and k == 0),
                                         stop=(c == NCH - 1 and k == 3),
                                         skip_group_check=True)

            def alloc_sc(st):
                st["sc"] = [psp.tile([P, 512], F32, tag="ps",
                                     name=f"sc{st['bi']}_{st['h']}_{i}")
                            for i in range(TP)]

            def AS_rest(st):
                """Interleave remaining A chunks (lead 2) with all S chunks."""
                for c in range(NCH):
                    if c + 2 < NCH:
                        A_chunks(st, [c + 2])
                    S_chunk(st, c)

            def C_finish(st):
                """Masked scaled copy of the packed strict-upper scores."""
                sc = st["sc"]
                for i in range(TP):
                    w_i = T - P * i
                    nc.vector.tensor_tensor(s_mask[:, SM_BASE[i]:SM_BASE[i] + P],
                                            sc[i][:, 0:P], umask[:], MUL)
                    if i < TP - 1:
                        nc.scalar.activation(s_mask[:, SM_BASE[i] + P:SM_BASE[i] + w_i],
                                             sc[i][:, P:w_i], AF.Copy, scale=SCALE)
                st["sc"] = None

            def D_LN(st):
                """attn = S_masked @ xi ; ykv = LN(attn); ykvT = transpose."""
                h, bi = st["h"], st["bi"]
                for i in range(TP):
                    pa = psp.tile([P, 512], F32, tag="ps", name=f"pa{bi}_{h}_{i}")
                    for k in range(i + 1):
                        off = SM_BASE[k] + P * (i - k)
                        nc.tensor.matmul(pa[:], s_mask[:, off:off + P], xi[:, k, :],
                                         start=(k == 0), stop=(k == i))
                    ln_tile(pa[:], ykv[:, i, :])
                for i in range(TP):
                    transpose_tile(ykv, ykvT, i)

            def FH(st):
                """y_sparse GEMM + xy product + decoder GEMM partials."""
                h, bi = st["h"], st["bi"]
                encv_d, dec_d = st["encv"], st["dec"]
                xsp = st["xsp"]
                srcv = encv_d[h].rearrange("(dt p) n -> p dt n", p=P)
                ydec = [psp.tile([P, 512], F32, tag="ps", name=f"yd{bi}_{h}_{i}")
                        for i in range(TP)]
                st["ydec"] = ydec
                base = h * N
                for c in range(NCH):
                    wv = wp.tile([P, DP, 512], BF16, tag="w")
                    nc.sync.dma_start(wv[:], srcv[:, :, 512 * c:512 * (c + 1)])
                    yst = ystp.tile([P, 4, T], BF16, tag="ys")
                    for j in range(4):
                        py = psp.tile([P, 512], F32, tag="ps", name=f"py{bi}_{h}_{c}_{j}")
                        for kt in range(DP):
                            nc.tensor.matmul(py[:], wv[:, kt, j * P:(j + 1) * P],
                                             ykvT[:, kt, :],
                                             start=(kt == 0), stop=(kt == DP - 1))
                        nc.scalar.activation(yst[:, j, :], py[:], AF.Relu)
                    nc.vector.tensor_tensor(xsp[:, 4 * c:4 * c + 4, :],
                                            xsp[:, 4 * c:4 * c + 4, :], yst[:], MUL)
                    wd = wp.tile([P, DP, 512], BF16, tag="w")
                    nc.sync.dma_start(
                        wd[:],
                        dec_d[base + 512 * c:base + 512 * (c + 1), :]
                        .rearrange("(kt p) d -> p kt d", p=P))
                    for i in range(TP):
                        for k in range(4):
                            nc.tensor.matmul(ydec[i][:], xsp[:, 4 * c + k, i * P:(i + 1) * P],
                                             wd[:, k, :],
                                             start=(c == 0 and k == 0),
                                             stop=(c == NCH - 1 and k == 3),
                                             skip_group_check=True)

            def flush(st, bi_dram):
                """Accumulate decoder partials into y_acc; DMA out on last head."""
                h = st["h"]
                for i in range(TP):
                    if h == 0:
                        nc.vector.tensor_copy(y_acc[:, i, :], st["ydec"][i][:])
                    else:
                        nc.vector.tensor_tensor(y_acc[:, i, :], y_acc[:, i, :],
                                                st["ydec"][i][:], ADD)
                    if h == NH - 1:
                        nc.sync.dma_start(
                            bi_dram.rearrange("(i p) d -> p i d", p=P)[:, i, :],
                            y_acc[:, i, :])
                st["ydec"] = None

            # ---- block ----
            def run_block(kind, bi):
                lk = kind.lower()
                x = z[kind]
                other = z["H" if kind == "L" else "L"]

                # prologue: xi = LN(x + inj), per-tile chains + transposes
                for i in range(TP):
                    nc.vector.tensor_tensor(y_acc[:, i, :], x[:, i, :], other[:, i, :], ADD)
                    if kind == "L":
                        nc.vector.tensor_tensor(y_acc[:, i, :], y_acc[:, i, :],
                                                inp[:, i, :], ADD)
                    ln_tile(y_acc[:, i, :], xi[:, i, :])
                    transpose_tile(xi, xiT, i)

                def new_state(h):
                    return {
                        "h": h, "bi": bi, "qrt": {}, "sc": None, "ydec": None,
                        "xsp": xspp.tile([P, NT, T], BF16, tag="xsp",
                                         name=f"xsp{bi}_{h}"),
                        "enc": w_enc[lk], "encv": w_encv[lk], "dec": w_dec[lk],
                    }

                bi_dram = dramp.tile([T, D], F32, tag="arbuf")

                states = [None] * NH
                states[0] = new_state(0)
                A_chunks(states[0], [0, 1], tsplit=True)
                alloc_sc(states[0])
                AS_rest(states[0])
                for h in range(NH):
                    st = states[h]
                    C_finish(st)
                    D_LN(st)
                    if h + 1 < NH:
                        states[h + 1] = new_state(h + 1)
                        A_chunks(states[h + 1], [0, 1])
                    FH(st)
                    flush(st, bi_dram)
                    if h + 1 < NH:
                        alloc_sc(states[h + 1])
                        AS_rest(states[h + 1])

                # ---- pairwise AllReduce of y partials ----
                nc.gpsimd.collective_compute(
                    "AllReduce", ADD,
                    replica_groups=[[0, 1], [2, 3], [4, 5], [6, 7]],
                    ins=[bi_dram.opt()],
                    outs=[bi_dram.opt()],
                )
                # yn = LN(y_sum); newz = LN(x + yn)  (per-tile chains)
                for i in range(TP):
                    nc.sync.dma_start(
                        y_acc[:, i, :],
                        bi_dram.rearrange("(i p) d -> p i d", p=P)[:, i, :])
                    ln_tile(y_acc[:, i, :], yn[:, i, :])
                    nc.vector.tensor_tensor(y_acc[:, i, :], x[:, i, :], yn[:, i, :], ADD)
                    ln_tile(y_acc[:, i, :], x[:, i, :])

            for bi in range(n_blocks):
                run_block(BLOCK_SEQ[bi], bi)

            if debug_z:
                last = z[BLOCK_SEQ[n_blocks - 1]]
                nc.sync.dma_start(z_out.rearrange("(i p) d -> p i d", p=P), last[:])
            else:
                # ---- lm_head: logits = z_H @ wlm (this core's V/2 slice) ----
                for i in range(TP):
                    nc.scalar.activation(yn[:, i, :], z["H"][:, i, :], AF.Copy)
                    transpose_tile(yn, xiT, i)  # xiT reused as zhT (bf16)
                for vc in range(8):
                    wlm_t = wp.tile([P, DP, 512], BF16, tag="w")
                    nc.sync.dma_start(
                        wlm_t[:],
                        wlm_d[:, 512 * vc:512 * (vc + 1)]
                        .rearrange("(kt p) v -> p kt v", p=P))
                    for i in range(TP):
                        pl = psp.tile([P, 512], F32, tag="ps", name=f"pl{vc}_{i}")
                        for kt in range(DP):
                            nc.tensor.matmul(pl[:], xiT[:, kt, i * P:(i + 1) * P],
                                             wlm_t[:, kt, :],
                                             start=(kt == 0), stop=(kt == DP - 1))
                        ot = ystp.tile([P, T], F32, tag="ys")
                        nc.scalar.activation(ot[:], pl[:], AF.Copy)
                        nc.sync.dma_start(
                            logits[i * P:(i + 1) * P, 512 * vc:512 * (vc + 1)], ot[:])

    nc.finalize()
    return nc


# ---------------- host side ----------------

# Chunk-interleaved latent permutation: chunk c holds original even latents
# 2k (k in [256c, 256c+256)) then odd latents 2k+1, so each chunk is one
# contiguous 512-column block = tiles (e,e,o,o).
_PERM = np.concatenate([
    np.concatenate([np.arange(512 * c, 512 * (c + 1), 2),
                    np.arange(512 * c + 1, 512 * (c + 1), 2)])
    for c in range(NCH)
])


def _rope_half_tables():
    """bf16 (2048, T) cos/sin tables, pair-deduped, transposed to [pair, t]."""
    q = np.floor(np.arange(N, dtype=np.float32) / 2.0) * 2.0
    freqs = (1.0 / np.power(np.float32(2.0 ** 16), q / np.float32(N))
             / np.float32(2 * math.pi)).astype(np.float32)
    f_even = freqs[0::2]  # (2048,)
    phases = np.arange(T, dtype=np.float32)[None, :] * f_even[:, None]  # (2048, T)
    ang = (phases % 1.0) * np.float32(2 * math.pi)
    ch = np.cos(ang).astype(ml_dtypes.bfloat16)
    sh = np.sin(ang).astype(ml_dtypes.bfloat16)
    return ch, sh


def make_in_maps(inputs):
    idx = np.asarray(inputs["idx"])
    embed_w = np.asarray(inputs["embed_w"], np.float32)
    lm_head = np.asarray(inputs["lm_head"], np.float32).astype(ml_dtypes.bfloat16)
    h_init = np.asarray(inputs["h_init"], np.float32)
    l_init = np.asarray(inputs["l_init"], np.float32)

    ch, sh = _rope_half_tables()
    umask = (np.arange(P)[:, None] < np.arange(P)[None, :]).astype(np.float32) * SCALE

    w = {}
    for k in ("l", "h"):
        enc = np.asarray(inputs[f"{k}_enc"], np.float32)[:, :, _PERM].astype(ml_dtypes.bfloat16)
        encv = np.asarray(inputs[f"{k}_enc_v"], np.float32)[:, :, _PERM].astype(ml_dtypes.bfloat16)
        dec = (np.asarray(inputs[f"{k}_dec"], np.float32)
               .reshape(NH_TOT, N, D)[:, _PERM, :].astype(ml_dtypes.bfloat16))
        w[k] = (enc, encv, dec)

    zh0 = np.ascontiguousarray(np.broadcast_to(h_init, (T, D)), dtype=np.float32)
    zl0 = np.ascontiguousarray(np.broadcast_to(l_init, (T, D)), dtype=np.float32)

    in_maps = []
    for c in range(NCORES):
        b, g = c // 2, c % 2
        hs = slice(4 * g, 4 * g + 4)
        emb = np.ascontiguousarray(embed_w[idx[b]], dtype=np.float32)
        m = {
            "emb": emb,
            "zh0": zh0,
            "zl0": zl0,
            "ch": np.ascontiguousarray(ch),
            "sh": np.ascontiguousarray(sh),
            "umask": umask,
            "wlm": np.ascontiguousarray(lm_head[:, g * (V // 2):(g + 1) * (V // 2)]),
        }
        for k in ("l", "h"):
            enc, encv, dec = w[k]
            m[f"{k}_enc"] = np.ascontiguousarray(enc[hs])
            m[f"{k}_encv"] = np.ascontiguousarray(encv[hs])
            m[f"{k}_dec"] = np.ascontiguousarray(dec[hs].reshape(NH * N, D))
        in_maps.append(m)
    return in_maps


_CACHE = {}


def _get_program(n_blocks=9, debug_z=False):
    key = (n_blocks, debug_z)
    if key not in _CACHE:
        _CACHE[key] = build_program(n_blocks, debug_z)
    return _CACHE[key]


def run(inputs, n_blocks=9, debug_z=False, trace=False, tmpdir=None):
    nc = _get_program(n_blocks, debug_z)
    in_maps = make_in_maps(inputs)
    r = bass_utils.run_bass_kernel_spmd(
        nc, in_maps, core_ids=list(range(NCORES)), trace=trace, tmpdir=tmpdir)
    return r


def kernel(**inputs) -> np.ndarray:
    r = run(inputs)
    out = np.empty((B, T, V), np.float32)
    for c in range(NCORES):
        b, g = c // 2, c % 2
        out[b][:, g * (V // 2):(g + 1) * (V // 2)] = r.results[c]["logits"]
    return out


# revision 16
# speedup vs baseline: 1.8620x; 1.1141x over previous
"""Trainium2 Bass kernel for nn_BDHHRM_23424751632641 (sparse_attention).

Contract: kernel(**inputs) takes FULL unsharded numpy inputs (as produced by
the problem's setup_inputs) and returns the FULL (B, T, V) fp32 logits.

Sharding: 8 cores = 4 batches x 2 head-groups. Core c handles batch c//2 and
heads [4*(c%2), 4*(c%2)+4). The decoder GEMM partial sums are combined with a
pairwise AllReduce ([[0,1],[2,3],[4,5],[6,7]]); everything else is local.

Device numerics: all GEMM operands in bf16 (fp32 PSUM accumulation), LN /
residual stream in fp32. End-to-end rel err ~1e-2 (sim-verified).

Schedule: software-pipelined head loop. Per head h the emission order is
  C-finish(h) -> D+LN(h) -> transpose(h) -> A_pre(h+1) -> F/H(h) -> flush(h)
  -> A+S interleaved rest(h+1)
so the PE fills LN/collective gaps of head h with head h+1's encoder GEMMs.
x_sparse is double-buffered across heads; scores PSUM accumulators (4 banks)
and decoder accumulators (4 banks) are never live simultaneously.
"""

import math
import sys

import numpy as np

sys.path.insert(0, "/opt/trn_rl_repo")

import ml_dtypes  # noqa: E402

import concourse.bass as bass  # noqa: E402
import concourse.tile as tile  # noqa: E402
from concourse import bacc, mybir, bass_utils  # noqa: E402
from concourse.masks import make_identity  # noqa: E402

# Problem shapes (hardcoded per contract)
B, T, D, V = 4, 512, 512, 8192
NH_TOT, N = 8, 4096
NH = 4          # heads per core
NCORES = 8
P = 128
TP = T // P     # 4 t-tiles
DP = D // P     # 4 d-tiles
NT = N // P     # 32 n-tiles per head
NCH = 8         # chunks per head; chunk c = tiles [4c,4c+4) = (e,e,o,o)
H_CYCLES, L_CYCLES = 3, 2
EPS = 1e-5
SCALE = float(N) ** -0.5

F32 = mybir.dt.float32
BF16 = mybir.dt.bfloat16
ADD = mybir.AluOpType.add
SUB = mybir.AluOpType.subtract
MUL = mybir.AluOpType.mult
AF = mybir.ActivationFunctionType

BLOCK_SEQ = ["L", "L", "H"] * 3
SM_BASE = [0, 512, 896, 1152]  # packed col base for score row-tile k (width 512-128k)
SM_W = 1280


def _dram_in(nc, name, shape, dt=F32):
    return nc.dram_tensor(name, list(shape), dt, kind="ExternalInput").ap()


def build_program(n_blocks=9, debug_z=False):
    nc = bacc.Bacc("TRN2", target_bir_lowering=False, debug=False, num_devices=NCORES)

    emb = _dram_in(nc, "emb", (T, D))
    zh0 = _dram_in(nc, "zh0", (T, D))
    zl0 = _dram_in(nc, "zl0", (T, D))
    w_enc = {k: _dram_in(nc, f"{k}_enc", (NH, D, N), BF16) for k in ("l", "h")}
    w_encv = {k: _dram_in(nc, f"{k}_encv", (NH, D, N), BF16) for k in ("l", "h")}
    w_dec = {k: _dram_in(nc, f"{k}_dec", (NH * N, D), BF16) for k in ("l", "h")}
    ch_d = _dram_in(nc, "ch", (N // 2, T), BF16)
    sh_d = _dram_in(nc, "sh", (N // 2, T), BF16)
    umask_d = _dram_in(nc, "umask", (P, P))
    wlm_d = _dram_in(nc, "wlm", (D, V // 2), BF16)

    if debug_z:
        z_out = nc.dram_tensor("z_out", [T, D], F32, kind="ExternalOutput").ap()
    else:
        logits = nc.dram_tensor("logits", [T, V // 2], F32, kind="ExternalOutput").ap()

    with tile.TileContext(nc) as tc:
        with (
            tc.tile_pool(name="pers", bufs=1) as pers,
            tc.tile_pool(name="xspp", bufs=2) as xspp,
            tc.tile_pool(name="qrtp", bufs=4) as qrtp,
            tc.tile_pool(name="rtp", bufs=2) as rtp,
            tc.tile_pool(name="wp", bufs=3) as wp,
            tc.tile_pool(name="ystp", bufs=2) as ystp,
            tc.tile_pool(name="stat", bufs=8) as statp,
            tc.tile_pool(name="ps", bufs=8, space="PSUM") as psp,
            tc.tile_pool(name="dram", bufs=1, space="DRAM") as dramp,
        ):
            # ---- persistent tiles ----
            ch_t = pers.tile([P, NT // 2, T], BF16, tag="ch")
            sh_t = pers.tile([P, NT // 2, T], BF16, tag="sh")
            z = {
                "L": pers.tile([P, TP, D], F32, tag="zl", name="z_L"),
                "H": pers.tile([P, TP, D], F32, tag="zh", name="z_H"),
            }
            inp = pers.tile([P, TP, D], BF16, tag="inp")
            xi = pers.tile([P, TP, D], BF16, tag="xi")
            xiT = pers.tile([P, DP, T], BF16, tag="xiT")
            ykv = pers.tile([P, TP, D], BF16, tag="ykv")
            ykvT = pers.tile([P, DP, T], BF16, tag="ykvT")
            y_acc = pers.tile([P, TP, D], F32, tag="yacc")
            yn = pers.tile([P, TP, D], BF16, tag="yn")
            s_mask = pers.tile([P, SM_W], BF16, tag="smask")
            ident = pers.tile([P, P], BF16, tag="ident")
            umask = pers.tile([P, P], F32, tag="umask")
            epsv = pers.tile([P, 1], F32, tag="epsv")
            nc.gpsimd.memset(epsv[:], EPS)

            make_identity(nc, ident)
            nc.sync.dma_start(umask[:], umask_d)
            nc.sync.dma_start(ch_t[:], ch_d.rearrange("(i p) t -> p i t", p=P))
            nc.sync.dma_start(sh_t[:], sh_d.rearrange("(i p) t -> p i t", p=P))
            nc.sync.dma_start(z["H"][:], zh0.rearrange("(i p) d -> p i d", p=P))
            nc.sync.dma_start(z["L"][:], zl0.rearrange("(i p) d -> p i d", p=P))

            # ---- helpers ----
            def ln_tile(src_ap, dst_ap):
                """dst = layernorm(src) along free dim (D)."""
                st6 = statp.tile([P, 6], F32, tag="st6")
                st2 = statp.tile([P, 2], F32, tag="st2")
                std = statp.tile([P, 1], F32, tag="std")
                rstd = statp.tile([P, 1], F32, tag="rstd")
                mb = statp.tile([P, 1], F32, tag="mb")
                nc.vector.bn_stats(st6[:], src_ap)
                nc.vector.bn_aggr(st2[:], st6[:])
                nc.scalar.activation(std[:], st2[:, 1:2], AF.Sqrt, bias=epsv[:])
                nc.vector.reciprocal(rstd[:], std[:])
                nc.vector.scalar_tensor_tensor(mb[:], st2[:, 0:1], -1.0, rstd[:], MUL, MUL)
                nc.scalar.activation(dst_ap, src_ap, AF.Identity, bias=mb[:], scale=rstd[:])

            def transpose_tile(src_tile, dst_tile, i):
                """dst[:, dj, i*P:(i+1)*P] = src[:, i, :].T for dj in 0..DP."""
                for dj in range(DP):
                    pt = psp.tile([P, P], BF16, tag="ps", name=f"tp_{i}_{dj}")
                    nc.tensor.transpose(pt[:], src_tile[:, i, dj * P:(dj + 1) * P], ident[:])
                    nc.scalar.activation(dst_tile[:, dj, i * P:(i + 1) * P], pt[:], AF.Copy)

            # ---- embedding prologue: inp = LN(emb) ----
            nc.sync.dma_start(y_acc[:], emb.rearrange("(i p) d -> p i d", p=P))
            for i in range(TP):
                ln_tile(y_acc[:, i, :], inp[:, i, :])

            # ---- per-head phase emitters ----
            def A_chunks(st, chunks, tsplit=False):
                """x_sparse GEMM + relu + rope for the given chunks."""
                h, bi, enc_d = st["h"], st["bi"], st["enc"]
                xsp = st["xsp"]
                src = enc_d[h].rearrange("(dt p) n -> p dt n", p=P)
                for c in chunks:
                    w = wp.tile([P, DP, 512], BF16, tag="w")
                    nc.sync.dma_start(w[:], src[:, :, 512 * c:512 * (c + 1)])
                    for j in range(4):
                        px = psp.tile([P, 512], F32, tag="ps", name=f"px{bi}_{h}_{c}_{j}")
                        if tsplit:
                            for it in range(TP):
                                for kt in range(DP):
                                    nc.tensor.matmul(
                                        px[:, it * P:(it + 1) * P],
                                        w[:, kt, j * P:(j + 1) * P],
                                        xiT[:, kt, it * P:(it + 1) * P],
                                        start=(kt == 0), stop=(kt == DP - 1),
                                        skip_group_check=True)
                        else:
                            for kt in range(DP):
                                nc.tensor.matmul(px[:], w[:, kt, j * P:(j + 1) * P],
                                                 xiT[:, kt, :],
                                                 start=(kt == 0), stop=(kt == DP - 1))
                        nc.scalar.activation(xsp[:, 4 * c + j, :], px[:], AF.Relu)
                    qrt = qrtp.tile([P, 4, T], BF16, tag="qrt")
                    st["qrt"][c] = qrt
                    e2 = xsp[:, 4 * c:4 * c + 2, :]
                    o2 = xsp[:, 4 * c + 2:4 * c + 4, :]
                    ch2 = ch_t[:, 2 * c:2 * c + 2, :]
                    sh2 = sh_t[:, 2 * c:2 * c + 2, :]
                    t1 = rtp.tile([P, 2, T], BF16, tag="rt")
                    t2 = rtp.tile([P, 2, T], BF16, tag="rt")
                    nc.vector.tensor_tensor(t1[:], o2, sh2, MUL)
                    nc.vector.tensor_tensor(qrt[:, 0:2, :], e2, ch2, MUL)
                    nc.vector.tensor_tensor(qrt[:, 0:2, :], qrt[:, 0:2, :], t1[:], SUB)
                    nc.vector.tensor_tensor(t2[:], e2, sh2, MUL)
                    nc.vector.tensor_tensor(qrt[:, 2:4, :], o2, ch2, MUL)
                    nc.vector.tensor_tensor(qrt[:, 2:4, :], qrt[:, 2:4, :], t2[:], ADD)

            def S_chunk(st, c):
                """Scores contribution of chunk c into the 4 packed accumulators."""
                qrt = st["qrt"].pop(c)
                sc = st["sc"]
                for i in range(TP):
                    t_lo = P * i
                    w_sc = T - t_lo
                    for k in range(4):
                        nc.tensor.matmul(sc[i][:, :w_sc],
                                         qrt[:, k, t_lo:t_lo + P],
                                         qrt[:, k, t_lo:],
                                         start=(c == 0 and k == 0),
                                         stop=(c == NCH - 1 and k == 3),
                                         skip_group_check=True)

            def alloc_sc(st):
                st["sc"] = [psp.tile([P, 512], F32, tag="ps",
                                     name=f"sc{st['bi']}_{st['h']}_{i}")
                            for i in range(TP)]

            def AS_rest(st):
                """Interleave remaining A chunks (lead 2) with all S chunks."""
                for c in range(NCH):
                    if c + 2 < NCH:
                        A_chunks(st, [c + 2])
                    S_chunk(st, c)

            def C_finish(st):
                """Masked scaled copy of the packed strict-upper scores."""
                sc = st["sc"]
                for i in range(TP):
                    w_i = T - P * i
                    nc.vector.tensor_tensor(s_mask[:, SM_BASE[i]:SM_BASE[i] + P],
                                            sc[i][:, 0:P], umask[:], MUL)
                    if i < TP - 1:
                        nc.scalar.activation(s_mask[:, SM_BASE[i] + P:SM_BASE[i] + w_i],
                                             sc[i][:, P:w_i], AF.Copy, scale=SCALE)
                st["sc"] = None

            def D_LN(st):
                """attn = S_masked @ xi ; ykv = LN(attn)."""
                h, bi = st["h"], st["bi"]
                for i in range(TP):
                    pa = psp.tile([P, 512], F32, tag="ps", name=f"pa{bi}_{h}_{i}")
                    for k in range(i + 1):
                        off = SM_BASE[k] + P * (i - k)
                        nc.tensor.matmul(pa[:], s_mask[:, off:off + P], xi[:, k, :],
                                         start=(k == 0), stop=(k == i))
                    ln_tile(pa[:], ykv[:, i, :])

            def TR(st):
                for i in range(TP):
                    transpose_tile(ykv, ykvT, i)

            def FH(st):
                """y_sparse GEMM + xy product + decoder GEMM partials."""
                h, bi = st["h"], st["bi"]
                encv_d, dec_d = st["encv"], st["dec"]
                xsp = st["xsp"]
                srcv = encv_d[h].rearrange("(dt p) n -> p dt n", p=P)
                ydec = [psp.tile([P, 512], F32, tag="ps", name=f"yd{bi}_{h}_{i}")
                        for i in range(TP)]
                st["ydec"] = ydec
                base = h * N
                for c in range(NCH):
                    wv = wp.tile([P, DP, 512], BF16, tag="w")
                    nc.sync.dma_start(wv[:], srcv[:, :, 512 * c:512 * (c + 1)])
                    wd = wp.tile([P, DP, 512], BF16, tag="w")
                    nc.sync.dma_start(
                        wd[:],
                        dec_d[base + 512 * c:base + 512 * (c + 1), :]
                        .rearrange("(kt p) d -> p kt d", p=P))
                    # half-split software pipeline: py/relu/mul of half x overlap
                    # the dec matmuls of the previous half -> no PE stall
                    for half in range(2):
                        yst = ystp.tile([P, 2, T], BF16, tag="ys")
                        for j2 in range(2):
                            j = 2 * half + j2
                            py = psp.tile([P, 512], F32, tag="ps",
                                          name=f"py{bi}_{h}_{c}_{j}")
                            for kt in range(DP):
                                nc.tensor.matmul(py[:], wv[:, kt, j * P:(j + 1) * P],
                                                 ykvT[:, kt, :],
                                                 start=(kt == 0), stop=(kt == DP - 1))
                            nc.scalar.activation(yst[:, j2, :], py[:], AF.Relu)
                        nc.vector.tensor_tensor(
                            xsp[:, 4 * c + 2 * half:4 * c + 2 * half + 2, :],
                            xsp[:, 4 * c + 2 * half:4 * c + 2 * half + 2, :],
                            yst[:], MUL)
                    for k in range(4):
                        for i in range(TP):
                            nc.tensor.matmul(ydec[i][:], xsp[:, 4 * c + k, i * P:(i + 1) * P],
                                             wd[:, k, :],
                                             start=(c == 0 and k == 0),
                                             stop=(c == NCH - 1 and k == 3),
                                             skip_group_check=True)

            def flush(st, bi_dram):
                """Accumulate decoder partials into y_acc; cast+DMA out on last head."""
                h = st["h"]
                for i in range(TP):
                    if h == 0:
                        nc.vector.tensor_copy(y_acc[:, i, :], st["ydec"][i][:])
                    else:
                        nc.vector.tensor_tensor(y_acc[:, i, :], y_acc[:, i, :],
                                                st["ydec"][i][:], ADD)
                    if h == NH - 1:
                        nc.scalar.activation(ykv[:, i, :], y_acc[:, i, :], AF.Copy)
                        nc.sync.dma_start(
                            bi_dram.rearrange("(i p) d -> p i d", p=P)[:, i, :],
                            ykv[:, i, :])
                st["ydec"] = None

            # ---- block ----
            def prologue0(kind):
                """xi = LN(x + inj) for the first block (later blocks fold this
                into the previous block's boundary chain)."""
                x = z[kind]
                other = z["H" if kind == "L" else "L"]
                for i in range(TP):
                    nc.vector.tensor_tensor(y_acc[:, i, :], x[:, i, :], other[:, i, :], ADD)
                    if kind == "L":
                        nc.vector.tensor_tensor(y_acc[:, i, :], y_acc[:, i, :],
                                                inp[:, i, :], ADD)
                    ln_tile(y_acc[:, i, :], xi[:, i, :])
                    transpose_tile(xi, xiT, i)

            def run_block(kind, bi, nk):
                lk = kind.lower()
                x = z[kind]
                other = z["H" if kind == "L" else "L"]

                def new_state(h):
                    return {
                        "h": h, "bi": bi, "qrt": {}, "sc": None, "ydec": None,
                        "xsp": xspp.tile([P, NT, T], BF16, tag="xsp",
                                         name=f"xsp{bi}_{h}"),
                        "enc": w_enc[lk], "encv": w_encv[lk], "dec": w_dec[lk],
                    }

                bi_dram = dramp.tile([T, D], BF16, tag="arbuf")

                states = [None] * NH
                states[0] = new_state(0)
                A_chunks(states[0], [0, 1], tsplit=True)
                alloc_sc(states[0])
                AS_rest(states[0])
                for h in range(NH):
                    st = states[h]
                    C_finish(st)
                    D_LN(st)
                    if h + 1 < NH:
                        states[h + 1] = new_state(h + 1)
                        A_chunks(states[h + 1], [0])
                    TR(st)
                    if h + 1 < NH:
                        A_chunks(states[h + 1], [1])
                    FH(st)
                    flush(st, bi_dram)
                    if h + 1 < NH:
                        alloc_sc(states[h + 1])
                        AS_rest(states[h + 1])

                # ---- pairwise AllReduce of y partials (bf16) ----
                nc.gpsimd.collective_compute(
                    "AllReduce", ADD,
                    replica_groups=[[0, 1], [2, 3], [4, 5], [6, 7]],
                    ins=[bi_dram.opt()],
                    outs=[bi_dram.opt()],
                )
                # precompute next block's (other + inp) on the idle gpsimd
                # engine while the collective is in flight (ykvT is dead here)
                have_pre = nk == "L"
                if have_pre:
                    for i in range(TP):
                        nc.gpsimd.tensor_tensor(ykvT[:, i, :], other[:, i, :],
                                                inp[:, i, :], ADD)
                # per-tile chain: yn = LN(y_sum); z = LN(x + yn);
                # then next block's xi = LN(z + inj) and its transposes
                for i in range(TP):
                    nc.sync.dma_start(
                        ykv[:, i, :],
                        bi_dram.rearrange("(i p) d -> p i d", p=P)[:, i, :])
                    ln_tile(ykv[:, i, :], yn[:, i, :])
                    nc.vector.tensor_tensor(y_acc[:, i, :], x[:, i, :], yn[:, i, :], ADD)
                    ln_tile(y_acc[:, i, :], x[:, i, :])
                    if nk is not None:
                        if have_pre:
                            nc.vector.tensor_tensor(y_acc[:, i, :], x[:, i, :],
                                                    ykvT[:, i, :], ADD)
                        else:  # nk == "H": inj = the z just updated (kind L)
                            nc.vector.tensor_tensor(y_acc[:, i, :], z[nk][:, i, :],
                                                    x[:, i, :], ADD)
                        ln_tile(y_acc[:, i, :], xi[:, i, :])
                        transpose_tile(xi, xiT, i)

            prologue0(BLOCK_SEQ[0])
            for bi in range(n_blocks):
                nk = BLOCK_SEQ[bi + 1] if bi + 1 < n_blocks else None
                run_block(BLOCK_SEQ[bi], bi, nk)

            if debug_z:
                last = z[BLOCK_SEQ[n_blocks - 1]]
                nc.sync.dma_start(z_out.rearrange("(i p) d -> p i d", p=P), last[:])
            else:
                # ---- lm_head: logits = z_H @ wlm (this core's V/2 slice) ----
                for i in range(TP):
                    nc.scalar.activation(yn[:, i, :], z["H"][:, i, :], AF.Copy)
                    transpose_tile(yn, xiT, i)  # xiT reused as zhT (bf16)
                for vc in range(8):
                    wlm_t = wp.tile([P, DP, 512], BF16, tag="w")
                    nc.sync.dma_start(
                        wlm_t[:],
                        wlm_d[:, 512 * vc:512 * (vc + 1)]
                        .rearrange("(kt p) v -> p kt v", p=P))
                    for i in range(TP):
                        pl = psp.tile([P, 512], F32, tag="ps", name=f"pl{vc}_{i}")
                        for kt in range(DP):
                            nc.tensor.matmul(pl[:], xiT[:, kt, i * P:(i + 1) * P],
                                             wlm_t[:, kt, :],
                                             start=(kt == 0), stop=(kt == DP - 1))
                        ot = ystp.tile([P, T], F32, tag="ys")
                        nc.scalar.activation(ot[:], pl[:], AF.Copy)
                        nc.sync.dma_start(
                            logits[i * P:(i + 1) * P, 512 * vc:512 * (vc + 1)], ot[:])

    nc.finalize()
    return nc


# ---------------- host side ----------------

# Chunk-interleaved latent permutation: chunk c holds original even latents
# 2k (k in [256c, 256c+256)) then odd latents 2k+1, so each chunk is one
# contiguous 512-column block = tiles (e,e,o,o).
_PERM = np.concatenate([
    np.concatenate([np.arange(512 * c, 512 * (c + 1), 2),
                    np.arange(512 * c + 1, 512 * (c + 1), 2)])
    for c in range(NCH)
])


def _rope_half_tables():
    """bf16 (2048, T) cos/sin tables, pair-deduped, transposed to [pair, t]."""
    q = np.floor(np.arange(N, dtype=np.float32) / 2.0) * 2.0
    freqs = (1.0 / np.power(np.float32(2.0 ** 16), q / np.float32(N))
             / np.float32(2 * math.pi)).astype(np.float32)
    f_even = freqs[0::2]  # (2048,)
    phases = np.arange(T, dtype=np.float32)[None, :] * f_even[:, None]  # (2048, T)
    ang = (phases % 1.0) * np.float32(2 * math.pi)
    ch = np.cos(ang).astype(ml_dtypes.bfloat16)
    sh = np.sin(ang).astype(ml_dtypes.bfloat16)
    return ch, sh


def make_in_maps(inputs):
    idx = np.asarray(inputs["idx"])
    embed_w = np.asarray(inputs["embed_w"], np.float32)
    lm_head = np.asarray(inputs["lm_head"], np.float32).astype(ml_dtypes.bfloat16)
    h_init = np.asarray(inputs["h_init"], np.float32)
    l_init = np.asarray(inputs["l_init"], np.float32)

    ch, sh = _rope_half_tables()
    umask = (np.arange(P)[:, None] < np.arange(P)[None, :]).astype(np.float32) * SCALE

    w = {}
    for k in ("l", "h"):
        enc = np.asarray(inputs[f"{k}_enc"], np.float32)[:, :, _PERM].astype(ml_dtypes.bfloat16)
        encv = np.asarray(inputs[f"{k}_enc_v"], np.float32)[:, :, _PERM].astype(ml_dtypes.bfloat16)
        dec = (np.asarray(inputs[f"{k}_dec"], np.float32)
               .reshape(NH_TOT, N, D)[:, _PERM, :].astype(ml_dtypes.bfloat16))
        w[k] = (enc, encv, dec)

    zh0 = np.ascontiguousarray(np.broadcast_to(h_init, (T, D)), dtype=np.float32)
    zl0 = np.ascontiguousarray(np.broadcast_to(l_init, (T, D)), dtype=np.float32)

    in_maps = []
    for c in range(NCORES):
        b, g = c // 2, c % 2
        hs = slice(4 * g, 4 * g + 4)
        emb = np.ascontiguousarray(embed_w[idx[b]], dtype=np.float32)
        m = {
            "emb": emb,
            "zh0": zh0,
            "zl0": zl0,
            "ch": np.ascontiguousarray(ch),
            "sh": np.ascontiguousarray(sh),
            "umask": umask,
            "wlm": np.ascontiguousarray(lm_head[:, g * (V // 2):(g + 1) * (V // 2)]),
        }
        for k in ("l", "h"):
            enc, encv, dec = w[k]
            m[f"{k}_enc"] = np.ascontiguousarray(enc[hs])
            m[f"{k}_encv"] = np.ascontiguousarray(encv[hs])
            m[f"{k}_dec"] = np.ascontiguousarray(dec[hs].reshape(NH * N, D))
        in_maps.append(m)
    return in_maps


_CACHE = {}


def _get_program(n_blocks=9, debug_z=False):
    key = (n_blocks, debug_z)
    if key not in _CACHE:
        _CACHE[key] = build_program(n_blocks, debug_z)
    return _CACHE[key]


def run(inputs, n_blocks=9, debug_z=False, trace=False, tmpdir=None):
    nc = _get_program(n_blocks, debug_z)
    in_maps = make_in_maps(inputs)
    r = bass_utils.run_bass_kernel_spmd(
        nc, in_maps, core_ids=list(range(NCORES)), trace=trace, tmpdir=tmpdir)
    return r


def kernel(**inputs) -> np.ndarray:
    r = run(inputs)
    out = np.empty((B, T, V), np.float32)
    for c in range(NCORES):
        b, g = c // 2, c % 2
        out[b][:, g * (V // 2):(g + 1) * (V // 2)] = r.results[c]["logits"]
    return out
